# revision 9
# baseline (speedup 1.0000x reference)
"""Trainium2 Bass kernel for nn_LR_23029614641373 (embedding_lookup).

out[i] = [1-p, p],  p = sigmoid(w[u_i] + w[N_USERS + m_i] + b)
x: [B, 2] int (u, m), W: [1, 9923] f32, b: [1] f32, B = 4194304.

Strategy: pure data parallel over 8 NeuronCores (batch sharded), the
9923-entry table replicated into all 128 SBUF partitions on each core.
The lookups run on GPSIMD via InstAPGather (8 independent streams, one
per 16-partition group).  ap_gather is the binding constraint of this
whole problem: ~27.1 ns per index per Q7 stream (hardware-measured;
RD_CMD round-trip latency with Cayman ReadOverlap=0 — independent of
num_idxs, index values, and table size), i.e. ~3.4 ns per lookup per
NeuronCore with all 8 streams busy -> 2*524288 lookups ~= 3.55 ms.
Every other on-chip lookup mechanism is worse (indirect DMA: 1 index
per slowest AP dim; dma_gather: 256B rows; DVE/ACT: no data-dependent
addressing; PE one-hot: PSUM's 128-partition batch limit).

So the kernel's job is to keep everything else strictly inside the
gather shadow.  The gathered stream is kept *replicated* across each
group's 16 partitions: the pair-add and the two sigmoids are computed
redundantly on the (otherwise idle) Vector/Scalar engines, and the
output DMA simply reads partition 16g for group g — one contiguous
descriptor per group per tile.  This removes the SBUF->SBUF compaction
DMAs and the 128-byte output descriptors of an earlier revision (41.5k
descriptors -> ~2.2k; HWDGE generates ~1 descriptor per ~100 ns).

Measured cost model per tile: 221.7 us of pure gather (8192 idx) plus
~1x the duration of any DVE op overlapping the gather window (the Q7
cores share their SBUF port with the Vector engine), so the pair-add
stays a single 2-port tensor_add (4.5 us; a 1-port tensor_reduce runs
2x as long and costs more total) and writes its result to PSUM, which
keeps the DVE's write traffic off the shared SBUF port.  ACT and DMA
traffic use separate ports and hide completely.  Startup: the ap_gather
ucode library load takes ~220 us (NRT's Q7 IRAM loader) and is the
whole pipeline head; a dependency-free warm-up gather hoists it to
t~=7 us so the table broadcast and first input tiles load underneath
it.  The table loads on the scalar HWDGE ring so its 129 descriptors
generate concurrently with the sync-ring X-load; the last 32768 rows
run as two half tiles to halve the pipeline tail.  Measured: 3.643 ms
(vs 4.376 ms for the staged baseline in the same environment); pure
gather floor is 131072 idx/Q7-stream x 27.06 ns = 3.547 ms.
"""

import numpy as np

N_USERS = 6040
N_MOVIES = 3883
TABLE = N_USERS + N_MOVIES  # 9923
B_TOTAL = 4194304
N_CORES = 8
ROWS_PER_CORE = B_TOTAL // N_CORES  # 524288

TILE_ROWS = 32768           # rows per full tile
N_FULL = 15                 # full tiles; the last 32768 rows = 2 half tiles

_compiled = None

# Set by test harnesses: TRACE=True requests an NTFF profile; the full
# BassKernelResults of the last run is stashed in LAST_RESULTS.
TRACE = False
LAST_RESULTS = None


def _build(rows_per_core):
    import concourse.bacc as bacc
    import concourse.tile as tile
    from concourse import mybir

    assert rows_per_core == (N_FULL + 1) * TILE_ROWS

    nc = bacc.Bacc()
    f32 = mybir.dt.float32
    i32 = mybir.dt.int32
    i16 = mybir.dt.int16

    x_d = nc.dram_tensor("x", [rows_per_core, 2], i32, kind="ExternalInput")
    w_d = nc.dram_tensor("w", [1, TABLE], f32, kind="ExternalInput")
    b_d = nc.dram_tensor("b", [1, 1], f32, kind="ExternalInput")
    y_d = nc.dram_tensor("y", [rows_per_core, 2], f32, kind="ExternalOutput")

    # DRAM views: full tiles (t < N_FULL) and half tiles (s = 2*N_FULL..)
    # input: partition p holds rows [tile_base + p*rp, +rp) (contiguous)
    x_t = x_d.rearrange("(t p r) c -> t p (r c)", p=128, r=TILE_ROWS // 128)
    y_t = y_d.rearrange("(t g r) c -> t g (r c)", g=8, r=TILE_ROWS // 8)
    x_h = x_d.rearrange("(s p r) c -> s p (r c)", p=128, r=TILE_ROWS // 256)
    y_h = y_d.rearrange("(s g r) c -> s g (r c)", g=8, r=TILE_ROWS // 16)

    with tile.TileContext(nc) as tc:
        with (
            tc.tile_pool(name="const", bufs=1) as constp,
            tc.tile_pool(name="xp", bufs=2) as xp,
            tc.tile_pool(name="ip", bufs=2) as ipool,
            tc.tile_pool(name="vp", bufs=2) as vp,
            tc.tile_pool(name="ap", bufs=1, space="PSUM") as apool,
            tc.tile_pool(name="fp", bufs=2) as fp,
        ):
            # GPSIMD library warm-up: the ap_gather ucode library load
            # (MODIFY_POOL_CONFIG -> Q7 IRAM DMA) takes ~220 us and the
            # scheduler otherwise orders it after the table broadcast's
            # semaphore wait.  A no-op gather whose operands come from
            # vector-engine memsets (no DMA dependency) hoists the load
            # to t~=6 us so it overlaps the table/input DMAs.
            W0 = constp.tile([128, 1], f32)
            nc.vector.memset(W0[:, :], 0.0)
            I0 = constp.tile([128, 32], i16)
            nc.vector.memset(I0[:, :], 0)
            V0 = constp.tile([128, 32], f32)
            nc.gpsimd.ap_gather(V0[:, :], W0[:, :], I0[:, :], 128, 1, 1, 32)

            # Table replicated across all 128 partitions + bias scalars.
            # Issued on the scalar HWDGE ring so the descriptors generate
            # concurrently with the first X-load (sync ring).  The
            # replication runs as HBM->1 partition (40 KB) followed by an
            # SBUF->SBUF doubling tree: a direct to_broadcast reads the
            # 40 KB table from HBM 128 times (5 MB), and that HBM traffic
            # slows the concurrent (latency-bound) Q7 library load by
            # ~16 us.  The tree keeps startup HBM traffic negligible.
            Wt = constp.tile([128, TABLE], f32)
            nc.scalar.dma_start(out=Wt[0:1, :], in_=w_d[0:1, :])
            p = 1
            while p < 128:
                nc.scalar.dma_start(out=Wt[p : 2 * p, :], in_=Wt[0:p, :])
                p *= 2
            Bb = constp.tile([128, 1], f32)
            nc.scalar.dma_start(out=Bb[:, :], in_=b_d[0:1, :].to_broadcast((128, 1)))
            nB = constp.tile([128, 1], f32)
            nc.vector.tensor_scalar_mul(nB[:, :], Bb[:, :], -1.0)

            def do_tile(x_view, y_view, rows):
                """One tile: rows must be divisible by 256."""
                rp = rows // 128      # rows per partition
                gr = rows // 8        # rows per group
                ni = 2 * gr           # ap_gather stream length per group

                # Load x tile: [128, rp, 2] int32 (contiguous per partition)
                X = xp.tile([128, rp, 2], i32)
                nc.sync.dma_start(out=X[:, :, :], in_=x_view)

                # Index prep: I[p, d] = u_d, I[p, rp+d] = N_USERS + m_d
                I = ipool.tile([128, 2 * rp], i16)
                nc.vector.tensor_copy(I[:, 0:rp], X[:, :, 0])
                nc.vector.tensor_scalar_add(I[:, rp : 2 * rp], X[:, :, 1], N_USERS)

                # Gather: group g stream slot i reads I[16g + i%16, i//16]
                # -> V[p, d*16 + q] = w[u of row q*rp+d]      (i < gr)
                #    V[p, gr + d*16 + q] = w[m' of row q*rp+d] (i >= gr)
                # replicated across the 16 partitions of each group.
                V = vp.tile([128, ni], f32)
                nc.gpsimd.ap_gather(V[:, :], Wt[:, :], I[:, :], 128, TABLE, 1, ni)

                # Pair add in stream order: A[p, j] = logit of row
                # (j%16)*rp + j//16 of group p//16.  Keep as a 2-port
                # tensor_add: it overlaps the next gather and the Q7s
                # share their SBUF port with the DVE, but the 1-port
                # reduce alternative runs 2x as long and costs more.
                A = apool.tile([128, gr], f32)
                nc.vector.tensor_add(A[:, :], V[:, 0:gr], V[:, gr:ni])

                # Sigmoid + row-order transpose: F[p, 2*(q*rp+d)+c].
                # in-AP A[p, d*16+q] viewed as [p, q, d] (strides 1, 16).
                F = fp.tile([128, 2 * gr], f32)
                F4 = F.rearrange("p (q d c) -> p q d c", q=16, d=rp, c=2)
                A3 = A.rearrange("p (d q) -> p q d", q=16)
                nc.scalar.activation(
                    F4[:, :, :, 1], A3[:, :, :],
                    mybir.ActivationFunctionType.Sigmoid,
                    bias=Bb[:, 0:1], scale=1.0,
                )
                nc.scalar.activation(
                    F4[:, :, :, 0], A3[:, :, :],
                    mybir.ActivationFunctionType.Sigmoid,
                    bias=nB[:, 0:1], scale=-1.0,
                )

                # Output: group g rows from partition 16g only — 8 fat
                # descriptors on the scalar HWDGE ring (overlaps the
                # sync-ring input descriptor generation).
                Fg = F.rearrange("(g q) f -> q g f", q=16)
                nc.scalar.dma_start(out=y_view, in_=Fg[0])

            for t in range(N_FULL):
                do_tile(x_t[t], y_t[t], TILE_ROWS)
            # Last 32768 rows as two half tiles: halves the post-gather
            # tail (add + 2 sigmoids + out-DMA) left exposed at the end.
            do_tile(x_h[2 * N_FULL], y_h[2 * N_FULL], TILE_ROWS // 2)
            do_tile(x_h[2 * N_FULL + 1], y_h[2 * N_FULL + 1], TILE_ROWS // 2)

    nc.compile()
    return nc


def _get_compiled():
    global _compiled
    if _compiled is None:
        _compiled = _build(ROWS_PER_CORE)
    return _compiled


def kernel(x, W, b):
    from concourse.bass_utils import run_bass_kernel_spmd

    x = np.asarray(x)
    orig_rows = x.shape[0]
    assert x.shape == (B_TOTAL, 2), x.shape
    x32 = np.ascontiguousarray(x.astype(np.int32, copy=False))
    w = np.ascontiguousarray(np.asarray(W, dtype=np.float32).reshape(1, TABLE))
    bb = np.ascontiguousarray(np.asarray(b, dtype=np.float32).reshape(1, 1))

    nc = _get_compiled()
    in_maps = [
        {
            "x": x32[k * ROWS_PER_CORE : (k + 1) * ROWS_PER_CORE],
            "w": w,
            "b": bb,
        }
        for k in range(N_CORES)
    ]
    global LAST_RESULTS
    res = run_bass_kernel_spmd(nc, in_maps, list(range(N_CORES)), trace=TRACE)
    LAST_RESULTS = res
    out = np.concatenate([res.results[k]["y"] for k in range(N_CORES)], axis=0)
    assert out.shape == (orig_rows, 2)
    return out


# revision 12
# speedup vs baseline: 1.0029x; 1.0029x over previous
"""Trainium2 Bass kernel for nn_LR_23029614641373 (embedding_lookup).

out[i] = [1-p, p],  p = sigmoid(w[u_i] + w[N_USERS + m_i] + b)
x: [B, 2] int (u, m), W: [1, 9923] f32, b: [1] f32, B = 4194304.

Strategy: pure data parallel over 8 NeuronCores (batch sharded), the
9923-entry table replicated into all 128 SBUF partitions on each core.
The lookups run on GPSIMD via InstAPGather (8 independent streams, one
per 16-partition group).  ap_gather is the binding constraint of this
whole problem: ~27.1 ns per index per Q7 stream (hardware-measured;
RD_CMD round-trip latency with Cayman ReadOverlap=0 — independent of
num_idxs, index values, and table size), i.e. ~3.4 ns per lookup per
NeuronCore with all 8 streams busy -> 2*524288 lookups ~= 3.55 ms.
Every other on-chip lookup mechanism is worse (indirect DMA: 1 index
per slowest AP dim; dma_gather: 256B rows; DVE/ACT: no data-dependent
addressing; PE one-hot: PSUM's 128-partition batch limit).

So the kernel's job is to keep everything else strictly inside the
gather shadow.  The gathered stream is kept *replicated* across each
group's 16 partitions: the pair-add and the two sigmoids are computed
redundantly on the (otherwise idle) Vector/Scalar engines, and the
output DMA simply reads partition 16g for group g — one contiguous
descriptor per group per tile.  This removes the SBUF->SBUF compaction
DMAs and the 128-byte output descriptors of an earlier revision (41.5k
descriptors -> ~2.2k; HWDGE generates ~1 descriptor per ~100 ns).

Measured cost model per tile: 221.7 us of pure gather (8192 idx) plus
~1x the duration of any DVE op overlapping the gather window (the Q7
cores share their SBUF port with the Vector engine), so the pair-add
stays a single 2-port tensor_add (4.5 us; a 1-port tensor_reduce runs
2x as long and costs more total) and writes its result to PSUM, which
keeps the DVE's write traffic off the shared SBUF port.  ACT and DMA
traffic use separate ports and hide completely.  Startup: the ap_gather
ucode library load takes ~220 us (NRT's Q7 IRAM loader) and is the
whole pipeline head; a dependency-free warm-up gather hoists it to
t~=7 us so the table broadcast and first input tiles load underneath
it.  The table loads on the scalar HWDGE ring so its 129 descriptors
generate concurrently with the sync-ring X-load; the last 32768 rows
run as two half tiles to halve the pipeline tail.  Measured: 3.643 ms
(vs 4.376 ms for the staged baseline in the same environment); pure
gather floor is 131072 idx/Q7-stream x 27.06 ns = 3.547 ms.
"""

import numpy as np

N_USERS = 6040
N_MOVIES = 3883
TABLE = N_USERS + N_MOVIES  # 9923
B_TOTAL = 4194304
N_CORES = 8
ROWS_PER_CORE = B_TOTAL // N_CORES  # 524288

TILE_ROWS = 32768           # rows per full tile
N_FULL = 15                 # full tiles; the last 32768 rows = 2 half tiles

_compiled = None

# Set by test harnesses: TRACE=True requests an NTFF profile; the full
# BassKernelResults of the last run is stashed in LAST_RESULTS.
TRACE = False
LAST_RESULTS = None


def _build(rows_per_core):
    import concourse.bacc as bacc
    import concourse.tile as tile
    from concourse import mybir

    assert rows_per_core == (N_FULL + 1) * TILE_ROWS

    nc = bacc.Bacc()
    f32 = mybir.dt.float32
    i32 = mybir.dt.int32
    i16 = mybir.dt.int16

    x_d = nc.dram_tensor("x", [rows_per_core, 2], i32, kind="ExternalInput")
    w_d = nc.dram_tensor("w", [1, TABLE], f32, kind="ExternalInput")
    b_d = nc.dram_tensor("b", [1, 1], f32, kind="ExternalInput")
    y_d = nc.dram_tensor("y", [rows_per_core, 2], f32, kind="ExternalOutput")

    # DRAM views: full tiles (t < N_FULL) and half tiles (s = 2*N_FULL..)
    # input: partition p holds rows [tile_base + p*rp, +rp) (contiguous)
    x_t = x_d.rearrange("(t p r) c -> t p (r c)", p=128, r=TILE_ROWS // 128)
    y_t = y_d.rearrange("(t g r) c -> t g (r c)", g=8, r=TILE_ROWS // 8)
    x_h = x_d.rearrange("(s p r) c -> s p (r c)", p=128, r=TILE_ROWS // 256)
    y_h = y_d.rearrange("(s g r) c -> s g (r c)", g=8, r=TILE_ROWS // 16)
    x_q = x_d.rearrange("(s p r) c -> s p (r c)", p=128, r=TILE_ROWS // 512)
    y_q = y_d.rearrange("(s g r) c -> s g (r c)", g=8, r=TILE_ROWS // 32)
    x_e = x_d.rearrange("(s p r) c -> s p (r c)", p=128, r=TILE_ROWS // 1024)
    y_e = y_d.rearrange("(s g r) c -> s g (r c)", g=8, r=TILE_ROWS // 64)

    with tile.TileContext(nc) as tc:
        with (
            tc.tile_pool(name="const", bufs=1) as constp,
            tc.tile_pool(name="xp", bufs=2) as xp,
            tc.tile_pool(name="ip", bufs=2) as ipool,
            tc.tile_pool(name="vp", bufs=2) as vp,
            tc.tile_pool(name="ap", bufs=1, space="PSUM") as apool,
            tc.tile_pool(name="fp", bufs=2) as fp,
        ):
            # GPSIMD library warm-up: the ap_gather ucode library load
            # (MODIFY_POOL_CONFIG -> Q7 IRAM DMA) takes ~220 us and the
            # scheduler otherwise orders it after the table broadcast's
            # semaphore wait.  A no-op gather whose operands come from
            # vector-engine memsets (no DMA dependency) hoists the load
            # to t~=6 us so it overlaps the table/input DMAs.
            W0 = constp.tile([128, 1], f32)
            nc.vector.memset(W0[:, :], 0.0)
            I0 = constp.tile([128, 32], i16)
            nc.vector.memset(I0[:, :], 0)
            V0 = constp.tile([128, 32], f32)
            nc.gpsimd.ap_gather(V0[:, :], W0[:, :], I0[:, :], 128, 1, 1, 32)

            # Table replicated across all 128 partitions + bias scalars.
            # Issued on the scalar HWDGE ring so these 129 descriptors
            # generate concurrently with the first X-load (sync ring).
            # (An SBUF->SBUF doubling-tree variant that avoids the 5 MB
            # of replicated HBM reads measured 43 us SLOWER end-to-end:
            # single-partition-source SBUF copies are port-bound.)
            Wt = constp.tile([128, TABLE], f32)
            nc.scalar.dma_start(out=Wt[:, :], in_=w_d[0:1, :].to_broadcast((128, TABLE)))
            Bb = constp.tile([128, 1], f32)
            nc.scalar.dma_start(out=Bb[:, :], in_=b_d[0:1, :].to_broadcast((128, 1)))
            nB = constp.tile([128, 1], f32)
            nc.vector.tensor_scalar_mul(nB[:, :], Bb[:, :], -1.0)

            def do_tile(x_view, y_view, rows):
                """One tile: rows must be divisible by 256."""
                rp = rows // 128      # rows per partition
                gr = rows // 8        # rows per group
                ni = 2 * gr           # ap_gather stream length per group

                # Load x tile: [128, rp, 2] int32 (contiguous per partition)
                X = xp.tile([128, rp, 2], i32)
                nc.sync.dma_start(out=X[:, :, :], in_=x_view)

                # Index prep: I[p, d] = u_d, I[p, rp+d] = N_USERS + m_d
                I = ipool.tile([128, 2 * rp], i16)
                nc.vector.tensor_copy(I[:, 0:rp], X[:, :, 0])
                nc.vector.tensor_scalar_add(I[:, rp : 2 * rp], X[:, :, 1], N_USERS)

                # Gather: group g stream slot i reads I[16g + i%16, i//16]
                # -> V[p, d*16 + q] = w[u of row q*rp+d]      (i < gr)
                #    V[p, gr + d*16 + q] = w[m' of row q*rp+d] (i >= gr)
                # replicated across the 16 partitions of each group.
                V = vp.tile([128, ni], f32)
                nc.gpsimd.ap_gather(V[:, :], Wt[:, :], I[:, :], 128, TABLE, 1, ni)

                # Pair add in stream order: A[p, j] = logit of row
                # (j%16)*rp + j//16 of group p//16.  Keep as a 2-port
                # tensor_add: it overlaps the next gather and the Q7s
                # share their SBUF port with the DVE, but the 1-port
                # reduce alternative runs 2x as long and costs more.
                A = apool.tile([128, gr], f32)
                nc.vector.tensor_add(A[:, :], V[:, 0:gr], V[:, gr:ni])

                # Sigmoid + row-order transpose: F[p, 2*(q*rp+d)+c].
                # in-AP A[p, d*16+q] viewed as [p, q, d] (strides 1, 16).
                F = fp.tile([128, 2 * gr], f32)
                F4 = F.rearrange("p (q d c) -> p q d c", q=16, d=rp, c=2)
                A3 = A.rearrange("p (d q) -> p q d", q=16)
                nc.scalar.activation(
                    F4[:, :, :, 1], A3[:, :, :],
                    mybir.ActivationFunctionType.Sigmoid,
                    bias=Bb[:, 0:1], scale=1.0,
                )
                nc.scalar.activation(
                    F4[:, :, :, 0], A3[:, :, :],
                    mybir.ActivationFunctionType.Sigmoid,
                    bias=nB[:, 0:1], scale=-1.0,
                )

                # Output: group g rows from partition 16g only — 8 fat
                # descriptors on the scalar HWDGE ring (overlaps the
                # sync-ring input descriptor generation).
                Fg = F.rearrange("(g q) f -> q g f", q=16)
                nc.scalar.dma_start(out=y_view, in_=Fg[0])

            for t in range(N_FULL):
                do_tile(x_t[t], y_t[t], TILE_ROWS)
            # Last 32768 rows as geometrically shrinking tiles (16K,
            # 8K, 4K, 4K): the post-gather tail (add + 2 sigmoids +
            # out-DMA) left exposed after the final gather scales with
            # the last tile, so it drops from ~25 us to ~4 us.
            do_tile(x_h[2 * N_FULL], y_h[2 * N_FULL], TILE_ROWS // 2)
            do_tile(x_q[4 * N_FULL + 2], y_q[4 * N_FULL + 2], TILE_ROWS // 4)
            do_tile(x_e[8 * N_FULL + 6], y_e[8 * N_FULL + 6], TILE_ROWS // 8)
            do_tile(x_e[8 * N_FULL + 7], y_e[8 * N_FULL + 7], TILE_ROWS // 8)

    nc.compile()
    return nc


def _get_compiled():
    global _compiled
    if _compiled is None:
        _compiled = _build(ROWS_PER_CORE)
    return _compiled


def kernel(x, W, b):
    from concourse.bass_utils import run_bass_kernel_spmd

    x = np.asarray(x)
    orig_rows = x.shape[0]
    assert x.shape == (B_TOTAL, 2), x.shape
    x32 = np.ascontiguousarray(x.astype(np.int32, copy=False))
    w = np.ascontiguousarray(np.asarray(W, dtype=np.float32).reshape(1, TABLE))
    bb = np.ascontiguousarray(np.asarray(b, dtype=np.float32).reshape(1, 1))

    nc = _get_compiled()
    in_maps = [
        {
            "x": x32[k * ROWS_PER_CORE : (k + 1) * ROWS_PER_CORE],
            "w": w,
            "b": bb,
        }
        for k in range(N_CORES)
    ]
    global LAST_RESULTS
    res = run_bass_kernel_spmd(nc, in_maps, list(range(N_CORES)), trace=TRACE)
    LAST_RESULTS = res
    out = np.concatenate([res.results[k]["y"] for k in range(N_CORES)], axis=0)
    assert out.shape == (orig_rows, 2)
    return out


# revision 14
# speedup vs baseline: 1.0119x; 1.0090x over previous
"""Trainium2 Bass kernel for nn_LR_23029614641373 (embedding_lookup).

out[i] = [1-p, p],  p = sigmoid(w[u_i] + w[N_USERS + m_i] + b)
x: [B, 2] int (u, m), W: [1, 9923] f32, b: [1] f32, B = 4194304.

Strategy: pure data parallel over 8 NeuronCores (batch sharded), the
9923-entry table replicated into all 128 SBUF partitions on each core.
The lookups run on GPSIMD via InstAPGather (8 independent streams, one
per 16-partition group).  ap_gather is the binding constraint of this
whole problem: ~27.1 ns per index per Q7 stream (hardware-measured;
RD_CMD round-trip latency with Cayman ReadOverlap=0 — independent of
num_idxs, index values, and table size), i.e. ~3.4 ns per lookup per
NeuronCore with all 8 streams busy -> 2*524288 lookups ~= 3.55 ms.
Every other on-chip lookup mechanism is worse (indirect DMA: 1 index
per slowest AP dim; dma_gather: 256B rows; DVE/ACT: no data-dependent
addressing; PE one-hot: PSUM's 128-partition batch limit).

So the kernel's job is to keep everything else strictly inside the
gather shadow.  The gathered stream is kept *replicated* across each
group's 16 partitions: the pair-add and the two sigmoids are computed
redundantly on the (otherwise idle) Vector/Scalar engines, and the
output DMA simply reads partition 16g for group g — one contiguous
descriptor per group per tile.  This removes the SBUF->SBUF compaction
DMAs and the 128-byte output descriptors of an earlier revision (41.5k
descriptors -> ~2.2k; HWDGE generates ~1 descriptor per ~100 ns).

Measured cost model per tile: 221.7 us of pure gather (8192 idx) plus
~1x the duration of any DVE op overlapping the gather window (the Q7
cores share their SBUF port with the Vector engine), so the pair-add
stays a single 2-port tensor_add (4.5 us; a 1-port tensor_reduce runs
2x as long and costs more total) and writes its result to PSUM, which
keeps the DVE's write traffic off the shared SBUF port.  ACT and DMA
traffic use separate ports and hide completely.  Startup: the ap_gather
ucode library load takes ~220 us (NRT's Q7 IRAM loader) and is the
whole pipeline head; a dependency-free warm-up gather hoists it to
t~=7 us so the table broadcast and first input tiles load underneath
it.  The table loads on the scalar HWDGE ring so its 129 descriptors
generate concurrently with the sync-ring X-load; the last 32768 rows
run as two half tiles to halve the pipeline tail.  Measured: 3.643 ms
(vs 4.376 ms for the staged baseline in the same environment); pure
gather floor is 131072 idx/Q7-stream x 27.06 ns = 3.547 ms.
"""

import numpy as np

N_USERS = 6040
N_MOVIES = 3883
TABLE = N_USERS + N_MOVIES  # 9923
B_TOTAL = 4194304
N_CORES = 8
ROWS_PER_CORE = B_TOTAL // N_CORES  # 524288

TILE_ROWS = 32768           # rows per full tile
N_FULL = 15                 # full tiles; the last 32768 rows = 2 half tiles

_compiled = None

# Set by test harnesses: TRACE=True requests an NTFF profile; the full
# BassKernelResults of the last run is stashed in LAST_RESULTS.
TRACE = False
LAST_RESULTS = None


def _build(rows_per_core):
    import concourse.bacc as bacc
    import concourse.tile as tile
    from concourse import mybir

    assert rows_per_core == (N_FULL + 1) * TILE_ROWS

    nc = bacc.Bacc()
    f32 = mybir.dt.float32
    i32 = mybir.dt.int32
    i16 = mybir.dt.int16

    x_d = nc.dram_tensor("x", [rows_per_core, 2], i32, kind="ExternalInput")
    w_d = nc.dram_tensor("w", [1, TABLE], f32, kind="ExternalInput")
    b_d = nc.dram_tensor("b", [1, 1], f32, kind="ExternalInput")
    y_d = nc.dram_tensor("y", [rows_per_core, 2], f32, kind="ExternalOutput")

    # DRAM views: full tiles (t < N_FULL) and half tiles (s = 2*N_FULL..)
    # input: partition p holds rows [tile_base + p*rp, +rp) (contiguous)
    x_t = x_d.rearrange("(t p r) c -> t p (r c)", p=128, r=TILE_ROWS // 128)
    y_t = y_d.rearrange("(t g r) c -> t g (r c)", g=8, r=TILE_ROWS // 8)
    x_h = x_d.rearrange("(s p r) c -> s p (r c)", p=128, r=TILE_ROWS // 256)
    y_h = y_d.rearrange("(s g r) c -> s g (r c)", g=8, r=TILE_ROWS // 16)

    with tile.TileContext(nc) as tc:
        with (
            tc.tile_pool(name="const", bufs=1) as constp,
            tc.tile_pool(name="xp", bufs=2) as xp,
            tc.tile_pool(name="ip", bufs=2) as ipool,
            tc.tile_pool(name="vp", bufs=2) as vp,
            tc.tile_pool(name="ap", bufs=1, space="PSUM") as apool,
            tc.tile_pool(name="fp", bufs=2) as fp,
        ):
            # GPSIMD library warm-up: the ap_gather ucode library load
            # (MODIFY_POOL_CONFIG -> Q7 IRAM DMA) takes ~220 us and the
            # scheduler otherwise orders it after the table broadcast's
            # semaphore wait.  A no-op gather whose operands come from
            # vector-engine memsets (no DMA dependency) hoists the load
            # to t~=6 us so it overlaps the table/input DMAs.
            W0 = constp.tile([128, 1], f32)
            nc.vector.memset(W0[:, :], 0.0)
            I0 = constp.tile([128, 32], i16)
            nc.vector.memset(I0[:, :], 0)
            V0 = constp.tile([128, 32], f32)
            nc.gpsimd.ap_gather(V0[:, :], W0[:, :], I0[:, :], 128, 1, 1, 32)

            # Table replicated across all 128 partitions + bias scalars.
            # Issued on the scalar HWDGE ring so these 129 descriptors
            # generate concurrently with the first X-load (sync ring).
            # (An SBUF->SBUF doubling-tree variant that avoids the 5 MB
            # of replicated HBM reads measured 43 us SLOWER end-to-end:
            # single-partition-source SBUF copies are port-bound.)
            Wt = constp.tile([128, TABLE], f32)
            nc.scalar.dma_start(out=Wt[:, :], in_=w_d[0:1, :].to_broadcast((128, TABLE)))
            Bb = constp.tile([128, 1], f32)
            nc.scalar.dma_start(out=Bb[:, :], in_=b_d[0:1, :].to_broadcast((128, 1)))
            nB = constp.tile([128, 1], f32)
            nc.vector.tensor_scalar_mul(nB[:, :], Bb[:, :], -1.0)

            def do_tile(x_view, y_view, rows):
                """One tile: rows must be divisible by 256."""
                rp = rows // 128      # rows per partition
                gr = rows // 8        # rows per group
                ni = 2 * gr           # ap_gather stream length per group

                # Load x tile: [128, rp, 2] int32 (contiguous per partition)
                X = xp.tile([128, rp, 2], i32)
                nc.sync.dma_start(out=X[:, :, :], in_=x_view)

                # Index prep: I[p, d] = u_d, I[p, rp+d] = N_USERS + m_d
                I = ipool.tile([128, 2 * rp], i16)
                nc.vector.tensor_copy(I[:, 0:rp], X[:, :, 0])
                nc.vector.tensor_scalar_add(I[:, rp : 2 * rp], X[:, :, 1], N_USERS)

                # Gather: group g stream slot i reads I[16g + i%16, i//16]
                # -> V[p, d*16 + q] = w[u of row q*rp+d]      (i < gr)
                #    V[p, gr + d*16 + q] = w[m' of row q*rp+d] (i >= gr)
                # replicated across the 16 partitions of each group.
                V = vp.tile([128, ni], f32)
                nc.gpsimd.ap_gather(V[:, :], Wt[:, :], I[:, :], 128, TABLE, 1, ni)

                # Pair add in stream order: A[p, j] = logit of row
                # (j%16)*rp + j//16 of group p//16.  Keep as a 2-port
                # tensor_add: it overlaps the next gather and the Q7s
                # share their SBUF port with the DVE, but the 1-port
                # reduce alternative runs 2x as long and costs more.
                A = apool.tile([128, gr], f32)
                nc.vector.tensor_add(A[:, :], V[:, 0:gr], V[:, gr:ni])

                # Sigmoid + row-order transpose: F[p, 2*(q*rp+d)+c].
                # in-AP A[p, d*16+q] viewed as [p, q, d] (strides 1, 16).
                F = fp.tile([128, 2 * gr], f32)
                F4 = F.rearrange("p (q d c) -> p q d c", q=16, d=rp, c=2)
                A3 = A.rearrange("p (d q) -> p q d", q=16)
                nc.scalar.activation(
                    F4[:, :, :, 1], A3[:, :, :],
                    mybir.ActivationFunctionType.Sigmoid,
                    bias=Bb[:, 0:1], scale=1.0,
                )
                nc.scalar.activation(
                    F4[:, :, :, 0], A3[:, :, :],
                    mybir.ActivationFunctionType.Sigmoid,
                    bias=nB[:, 0:1], scale=-1.0,
                )

                # Output: group g rows from partition 16g only — 8 fat
                # descriptors on the scalar HWDGE ring (overlaps the
                # sync-ring input descriptor generation).
                Fg = F.rearrange("(g q) f -> q g f", q=16)
                nc.scalar.dma_start(out=y_view, in_=Fg[0])

            for t in range(N_FULL):
                do_tile(x_t[t], y_t[t], TILE_ROWS)
            # Last 32768 rows as two half tiles: halves the post-gather
            # tail (add + 2 sigmoids + out-DMA) left exposed at the end.
            # (Shrinking further to quarter/eighth tiles measured 32 us
            # SLOWER: small ap_gather streams pay more per-instruction
            # overhead than the shorter tail saves.)
            do_tile(x_h[2 * N_FULL], y_h[2 * N_FULL], TILE_ROWS // 2)
            do_tile(x_h[2 * N_FULL + 1], y_h[2 * N_FULL + 1], TILE_ROWS // 2)

    nc.compile()
    return nc


def _get_compiled():
    global _compiled
    if _compiled is None:
        _compiled = _build(ROWS_PER_CORE)
    return _compiled


def kernel(x, W, b):
    from concourse.bass_utils import run_bass_kernel_spmd

    x = np.asarray(x)
    orig_rows = x.shape[0]
    assert x.shape == (B_TOTAL, 2), x.shape
    x32 = np.ascontiguousarray(x.astype(np.int32, copy=False))
    w = np.ascontiguousarray(np.asarray(W, dtype=np.float32).reshape(1, TABLE))
    bb = np.ascontiguousarray(np.asarray(b, dtype=np.float32).reshape(1, 1))

    nc = _get_compiled()
    in_maps = [
        {
            "x": x32[k * ROWS_PER_CORE : (k + 1) * ROWS_PER_CORE],
            "w": w,
            "b": bb,
        }
        for k in range(N_CORES)
    ]
    global LAST_RESULTS
    res = run_bass_kernel_spmd(nc, in_maps, list(range(N_CORES)), trace=TRACE)
    LAST_RESULTS = res
    out = np.concatenate([res.results[k]["y"] for k in range(N_CORES)], axis=0)
    assert out.shape == (orig_rows, 2)
    return out
